# revision 16
# baseline (speedup 1.0000x reference)
# Trainium2 Bass kernel for nn_FFF_v2 (fast-feedforward / MoE tree routing).
#
#   lam   = x @ W.T                      [B, 12] router logits
#   branch= lam > 0                      tree descent decisions
#   node  = (2^i - 1) + sum_{j<i} branch_j 2^(i-1-j)
#   out   = sum_i lam_i * Y[node_i]      [B, 4096]
#
# Sharding: data-parallel on batch across 8 cores (1024 rows each); W and Y
# replicated.  Per core:
#   - router logits via PE matmul in exact fp32 (branch signs must match the
#     fp32 reference; bf16 here would flip ~300 branches and fail absmax)
#   - tree-node ids via small exact matmuls (powers-of-two weights, fp32)
#   - levels 0..K_MM-1: scaled-one-hot matmul (bf16) against SBUF-resident
#     shallow Y rows -- exploits the massive node reuse at shallow levels
#   - levels K_MM..11: dma_gather of bf16 Y rows from HBM + per-partition
#     FMA chain split across the vector and gpsimd engines
#   - PSUM->SBUF copies of the one-hot result run on the Activation engine
#   - output written in bf16 (halves write traffic); host upcasts to fp32
#
# Schedule: software-pipelined macro emission  p1(0) p1(1) p2(0) p1(2) ...
# so the PE alternates router(m+1) / one-hot(m) without stalling, and the
# gather stream (Pool swdge) is never blocked behind index-build latency.
import numpy as np
import ml_dtypes

DEPTH = 12
B = 8192
D = 4096
N_NODES = 4095
NCORES = 8
B_LOC = B // NCORES          # 1024 rows per core

MACRO = 256                  # batch rows per macro tile
SUB = 128                    # rows per subtile (one partition block)
NSUB = MACRO // SUB          # 2
NMACRO = B_LOC // MACRO      # 4
NF = MACRO // 16             # 16-row wrap slots per macro

K_MM = 9                     # levels 0..K_MM-1 handled by one-hot matmul
N_SH = 2 ** K_MM - 1         # shallow nodes (511)
NCHUNK = (N_SH + 127) // 128  # 4
N_GL = DEPTH - K_MM          # gather levels (3)

_CACHE = {}


def _level_of(n):
    lev = 0
    while n >= 2 ** (lev + 1) - 1:
        lev += 1
    return lev


def _host_consts():
    # powT[j, i] = powmat[i, j] = 2^(i-1-j) for j < i  (lhsT of prefix matmul)
    powT = np.zeros((DEPTH, DEPTH), np.float32)
    for i in range(DEPTH):
        for j in range(i):
            powT[j, i] = float(1 << (i - 1 - j))
    # offs_w[p, l*NF+f] = 2^l - 1 (level offset in wrapped (level, slot) layout)
    offs = np.array([(1 << i) - 1 for i in range(DEPTH)], np.float32)
    offs_w = np.broadcast_to(np.repeat(offs, NF)[None, :], (16, DEPTH * NF)).copy()
    # bselT[l, c*128+p] = 1 if level(c*128+p) == l else 0   (lhsT of bc matmul)
    bselT = np.zeros((DEPTH, NCHUNK * 128), np.float32)
    # nrel[p, c] = node - (2^level - 1), or -1 for pad positions
    nrel = np.full((128, NCHUNK), -1.0, np.float32)
    for c in range(NCHUNK):
        for p in range(128):
            n = c * 128 + p
            if n < N_SH:
                lev = _level_of(n)
                bselT[lev, c * 128 + p] = 1.0
                nrel[p, c] = float(n - ((1 << lev) - 1))
    ident = np.eye(DEPTH, dtype=np.float32)
    return powT, offs_w, bselT, nrel, ident


def _build_program():
    import concourse.bass as bass
    import concourse.bacc as bacc
    import concourse.mybir as mybir
    import concourse.tile as tile
    from contextlib import ExitStack

    dt = mybir.dt
    f32 = dt.float32
    bf16 = dt.bfloat16
    i16 = dt.int16
    Alu = mybir.AluOpType

    nc = bacc.Bacc(trn_type="TRN2", num_swdge_queues=4)

    xt_d = nc.dram_tensor("xt", [NMACRO, 128, 32, MACRO], f32, kind="ExternalInput")
    y_d = nc.dram_tensor("y", [N_NODES, D], bf16, kind="ExternalInput")
    # zero-padded shallow Y rows (pad rows past N_SH must be 0.0, not garbage:
    # the one-hot matmul multiplies them by 0 and 0*NaN would poison PSUM)
    ysh_d = nc.dram_tensor("ysh", [NCHUNK * 128, D], bf16, kind="ExternalInput")
    wt_d = nc.dram_tensor("wt", [128, 32, DEPTH], f32, kind="ExternalInput")
    powt_d = nc.dram_tensor("powt", [DEPTH, DEPTH], f32, kind="ExternalInput")
    offsw_d = nc.dram_tensor("offsw", [16, DEPTH * NF], f32, kind="ExternalInput")
    bselt_d = nc.dram_tensor("bselt", [DEPTH, NCHUNK * 128], bf16, kind="ExternalInput")
    nrel_d = nc.dram_tensor("nrel", [128, NCHUNK], f32, kind="ExternalInput")
    ident_d = nc.dram_tensor("ident", [DEPTH, DEPTH], f32, kind="ExternalInput")
    out_d = nc.dram_tensor("out", [B_LOC, D], bf16, kind="ExternalOutput")

    with tile.TileContext(nc) as tc, ExitStack() as ctx:
        consts = ctx.enter_context(tc.tile_pool(name="consts", bufs=1))
        xt_p = ctx.enter_context(tc.tile_pool(name="xt", bufs=2))
        small = ctx.enter_context(tc.tile_pool(name="small", bufs=3))
        small4 = ctx.enter_context(tc.tile_pool(name="small4", bufs=4))
        st_p = ctx.enter_context(tc.tile_pool(name="st", bufs=2 * NCHUNK))
        g_p = ctx.enter_context(tc.tile_pool(name="g", bufs=8))
        out_p = ctx.enter_context(tc.tile_pool(name="outp", bufs=2))
        dram_p = ctx.enter_context(tc.tile_pool(name="idxd", bufs=2, space="DRAM"))
        ps_lam = ctx.enter_context(tc.tile_pool(name="pslam", bufs=1, space="PSUM"))
        ps_bc = ctx.enter_context(tc.tile_pool(name="psbc", bufs=2, space="PSUM"))
        ps_tp = ctx.enter_context(tc.tile_pool(name="pstp", bufs=1, space="PSUM"))
        ps_out = ctx.enter_context(tc.tile_pool(name="psout", bufs=3, space="PSUM"))

        # ---- critical-path constant: router weights only ----
        wt_sb = consts.tile([128, 32, DEPTH], f32)
        nc.sync.dma_start(wt_sb[:], wt_d.ap())

        # shallow Y rows: issued up-front on the (idle-until-gathers) Pool
        # software-DGE queue so they don't starve behind the xt stream on the
        # sync HWDGE queue (observed 58GB/s trickle + 90us gather delay when
        # these sat on the scalar HWDGE queue)
        ysh_sb = consts.tile([128, NCHUNK * D], bf16)
        for c in range(NCHUNK):
            nc.gpsimd.dma_start(
                ysh_sb[:, c * D : (c + 1) * D],
                ysh_d.ap()[c * 128 : (c + 1) * 128, :],
            )

        # small constants (loaded while the first router runs)
        powt_sb = consts.tile([DEPTH, DEPTH], f32)
        offsw_sb = consts.tile([16, DEPTH * NF], f32)
        bselt_sb = consts.tile([DEPTH, NCHUNK * 128], bf16)
        nrel_sb = consts.tile([128, NCHUNK], f32)
        ident_sb = consts.tile([DEPTH, DEPTH], f32)

        state = {}  # per-macro tiles handed from phase 1 to phase 2

        def phase1(m):
            # ---- load x^T macro tile [128, 32, MACRO] ----
            xt = xt_p.tile([128, 32, MACRO], f32, tag="xt")
            nc.sync.dma_start(xt[:, :16, :], xt_d.ap()[m][:, :16, :])
            nc.sync.dma_start(xt[:, 16:, :], xt_d.ap()[m][:, 16:, :])

            # ---- router: lam^T [12, MACRO] = W @ x^T  (exact fp32) ----
            lam_ps = ps_lam.tile([DEPTH, MACRO], f32, tag="lam")
            for c in range(32):
                nc.tensor.matmul(
                    lam_ps[:], wt_sb[:, c, :], xt[:, c, :],
                    start=(c == 0), stop=(c == 31),
                )

            if m == 0:
                nc.sync.dma_start(powt_sb[:], powt_d.ap())
                nc.sync.dma_start(offsw_sb[:], offsw_d.ap())
                nc.sync.dma_start(bselt_sb[:], bselt_d.ap())
                nc.sync.dma_start(nrel_sb[:], nrel_d.ap())
                nc.sync.dma_start(ident_sb[:], ident_d.ap())

            # branch bits, lam^T and prefix^T in SBUF (partition 0 based)
            branch = small.tile([DEPTH, MACRO], f32, tag="branch")
            nc.vector.tensor_scalar(branch[:], lam_ps[:], 0.0, None, Alu.is_gt)
            lamT = small.tile([DEPTH, MACRO], f32, tag="lamT")
            nc.scalar.copy(lamT[:], lam_ps[:])
            lamTb = small.tile([DEPTH, MACRO], bf16, tag="lamTb")
            nc.scalar.copy(lamTb[:], lam_ps[:])

            # prefix^T [12, MACRO] = powmat @ branch  (exact fp32; pb_ps
            # reuses lam_ps's PSUM buffer, so all lam_ps reads come first)
            pb_ps = ps_lam.tile([DEPTH, MACRO], f32, tag="lam")
            nc.tensor.matmul(pb_ps[:], powt_sb[:], branch[:], start=True, stop=True)
            pfxT = small.tile([DEPTH, MACRO], f32, tag="pfxT")
            nc.scalar.copy(pfxT[:], pb_ps[:])
            # bf16 copy for the bc matmuls (prefix values <= 255: exact)
            pfxTb = small.tile([DEPTH, MACRO], bf16, tag="pfxTb")
            nc.scalar.copy(pfxTb[:], pb_ps[:])

            # ---- S^T build: one chunk of 128 shallow nodes at a time ----
            st = []
            for c in range(NCHUNK):
                bc_ps = ps_bc.tile([128, 2 * MACRO], f32, tag="bc")
                nc.tensor.matmul(
                    bc_ps[:, :MACRO], bselt_sb[:, c * 128 : (c + 1) * 128],
                    pfxTb[:], start=True, stop=True,
                )
                nc.tensor.matmul(
                    bc_ps[:, MACRO:], bselt_sb[:, c * 128 : (c + 1) * 128],
                    lamTb[:], start=True, stop=True,
                )
                lbc = small.tile([128, MACRO], f32, tag="lbc")
                nc.scalar.copy(lbc[:], bc_ps[:, MACRO:])
                stc = st_p.tile([128, MACRO], bf16, tag="st")
                nc.vector.scalar_tensor_tensor(
                    stc[:], bc_ps[:, :MACRO], nrel_sb[:, c : c + 1], lbc[:],
                    Alu.is_equal, Alu.mult,
                )
                st.append(stc)

            # ---- node ids for the whole macro in the 16-partition-wrapped
            # (level, slot) layout dma_gather wants ----
            tpw = ps_tp.tile([16, NF * DEPTH], f32, tag="tpw")
            w_ps = tpw[:].rearrange("p (f l) -> p f l", f=NF)
            for f in range(NF):
                nc.tensor.matmul(
                    w_ps[:, f, :], pfxT[:, f * 16 : (f + 1) * 16], ident_sb[:],
                    start=True, stop=True,
                )
            idx16 = small4.tile([16, DEPTH, NF], i16, tag="idx16")
            nc.vector.tensor_tensor(
                idx16[:], w_ps[:].rearrange("p f l -> p l f"), offsw_sb[:], Alu.add
            )
            # replicate to all 8 Q7 descriptor-gen cores via a DRAM bounce
            # (on sync: tiny transfers, keeps the Pool queue pure gathers)
            idxd = dram_p.tile([16, N_GL * NF], i16, tag="idxd")
            nc.sync.dma_start(
                idxd[:], idx16[:, K_MM:, :].rearrange("p l f -> p (l f)")
            )
            idxr = small4.tile([128, N_GL, NF], i16, tag="idxr")
            for gq in range(8):
                nc.sync.dma_start(
                    idxr[16 * gq : 16 * (gq + 1), :, :].rearrange("p l f -> p (l f)"),
                    idxd[:],
                )

            # per-sub lam in batch-partition layout (bf16 scalars for FMA)
            lambs = []
            for s in range(NSUB):
                tp2 = ps_tp.tile([SUB, DEPTH], f32, tag="tp2")
                nc.tensor.matmul(
                    tp2[:], lamT[:, s * SUB : (s + 1) * SUB], ident_sb[:],
                    start=True, stop=True,
                )
                lamb = small4.tile([SUB, DEPTH], bf16, tag="lamb")
                nc.vector.tensor_copy(lamb[:], tp2[:])
                lambs.append(lamb)

            state[m] = (st, idxr, lambs)

        def phase2(m):
            st, idxr, lambs = state.pop(m)
            for s in range(NSUB):
                bsl = slice(s * SUB, (s + 1) * SUB)
                # ---- gather deep levels from HBM (bf16 rows) ----
                gt = []
                for li in range(N_GL):
                    g = g_p.tile([128, 1, D], bf16, tag="g")
                    nc.gpsimd.dma_gather(
                        g[:], y_d.ap(),
                        idxr[:, li, s * (SUB // 16) : (s + 1) * (SUB // 16)],
                        SUB, SUB, D,
                        queue_num=(m * NSUB * N_GL + s * N_GL + li) % 4,
                    )
                    gt.append(g)

                # ---- shallow one-hot matmul; PSUM->bf16 copies on Act ----
                out_t = out_p.tile([SUB, D], bf16, tag="out")
                for q in range(D // 512):
                    qsl = slice(q * 512, (q + 1) * 512)
                    po = ps_out.tile([SUB, 512], f32, tag="po")
                    for c in range(NCHUNK):
                        nc.tensor.matmul(
                            po[:], st[c][:, bsl],
                            ysh_sb[:, c * D + q * 512 : c * D + (q + 1) * 512],
                            start=(c == 0), stop=(c == NCHUNK - 1),
                        )
                    nc.scalar.copy(out_t[:, qsl], po[:])

                # ---- deep levels: per-partition FMA chain on DVE ----
                # (gpsimd TensorScalarPtr is rejected by the TRN2 ISA check)
                lamb = lambs[s]
                for li in range(N_GL):
                    eng = nc.vector
                    eng.scalar_tensor_tensor(
                        out_t[:], gt[li][:, 0, :],
                        lamb[:, K_MM + li : K_MM + li + 1],
                        out_t[:], Alu.mult, Alu.add,
                    )
                nc.scalar.dma_start(
                    out_d.ap()[m * MACRO + s * SUB :][:SUB, :], out_t[:]
                )

        # software-pipelined emission: p1(0) p1(1) p2(0) p1(2) p2(1) ...
        phase1(0)
        for m in range(1, NMACRO):
            phase1(m)
            phase2(m - 1)
        phase2(NMACRO - 1)

    nc.compile()
    return nc


def _patch_walrus_passes():
    # The default walrus pass list in this environment omits
    # lower_custom_kernel, which the Pool custom instructions (dma_gather)
    # need. Inject it in front of codegen.
    import concourse.bass_utils as bu

    if getattr(bu, "_ant_lck_patched", False):
        return
    bu._ant_lck_patched = True
    orig = bu.run_command

    def run_command(argv, **kw):
        if argv and "walrus_driver" in str(argv[0]):
            argv = list(argv)
            for i, a in enumerate(argv):
                if a == "--pass" and "lower_custom_kernel" not in argv[i + 1]:
                    argv[i + 1] = argv[i + 1].replace(
                        "codegen", "lower_custom_kernel,codegen"
                    )
                    break
        return orig(argv, **kw)

    bu.run_command = run_command


def _get_program():
    if "nc" not in _CACHE:
        _CACHE["nc"] = _build_program()
    return _CACHE["nc"]


def _prep_in_maps(x, W, Y):
    powT, offs_w, bselT, nrel, ident = _host_consts()
    Yb = np.ascontiguousarray(Y, np.float32).astype(ml_dtypes.bfloat16)
    ysh = np.zeros((NCHUNK * 128, D), ml_dtypes.bfloat16)
    ysh[:N_SH] = Yb[:N_SH]
    wt = np.ascontiguousarray(
        W.T.reshape(32, 128, DEPTH).transpose(1, 0, 2), np.float32
    )
    in_maps = []
    xr = x.reshape(NCORES, B_LOC, D)
    for c in range(NCORES):
        xt = xr[c].T  # [D, B_LOC]
        xtm = np.ascontiguousarray(
            xt.reshape(32, 128, NMACRO, MACRO).transpose(2, 1, 0, 3), np.float32
        )
        in_maps.append(
            {
                "xt": xtm, "y": Yb, "ysh": ysh, "wt": wt, "powt": powT,
                "offsw": offs_w, "bselt": bselT.astype(ml_dtypes.bfloat16),
                "nrel": nrel, "ident": ident,
            }
        )
    return in_maps


def kernel(x, W, Y, _trace=False):
    from concourse.bass_utils import run_bass_kernel_spmd

    _patch_walrus_passes()

    nc = _get_program()
    in_maps = _prep_in_maps(np.asarray(x), np.asarray(W), np.asarray(Y))
    res = run_bass_kernel_spmd(nc, in_maps, list(range(NCORES)), trace=_trace)
    out = np.concatenate(
        [np.asarray(res.results[c]["out"], dtype=np.float32) for c in range(NCORES)],
        axis=0,
    )
    if _trace:
        _CACHE["last_result"] = res
    return out


# revision 17
# speedup vs baseline: 1.1676x; 1.1676x over previous
# Trainium2 Bass kernel for nn_FFF_v2 (fast-feedforward / MoE tree routing).
#
#   lam   = x @ W.T                      [B, 12] router logits
#   branch= lam > 0                      tree descent decisions
#   node  = (2^i - 1) + sum_{j<i} branch_j 2^(i-1-j)
#   out   = sum_i lam_i * Y[node_i]      [B, 4096]
#
# Sharding: data-parallel on batch across 8 cores (1024 rows each); W and Y
# replicated.  Per core:
#   - router logits via PE matmul in exact fp32 (branch signs must match the
#     fp32 reference; bf16 here would flip ~300 branches and fail absmax)
#   - tree-node ids via small exact matmuls (powers-of-two weights, fp32)
#   - levels 0..K_MM-1: scaled-one-hot matmul (bf16) against SBUF-resident
#     shallow Y rows -- exploits the massive node reuse at shallow levels
#   - levels K_MM..11: dma_gather of fp8(e3m4, x128) Y rows from HBM +
#     per-partition FMA chain on the vector engine (lam scalars carry /128)
#   - PSUM->SBUF copies of the one-hot result run on the Activation engine
#   - output written in bf16 (halves write traffic); host upcasts to fp32
#
# Emission schedule (software pipelined, engine-queue aware):
#   A(m) = xt + router;  B(m) = branch/prefix/one-hot-build/index;
#   C(m,s) = gathers + one-hot matmul + FMA + store for one 128-row sub.
#   A0 B0 A1 [C(0,0) B1 C(0,1) A2] [C(1,0) B2 C(1,1) A3] [C(2,0) B3 C(2,1)]
#   C(3,0) C(3,1)
# keeps PE dense (router | small MMs | one-hot alternate) and never queues a
# blocked small op in front of ready FMA/gather work on DVE/Pool.
import numpy as np
import ml_dtypes

DEPTH = 12
B = 8192
D = 4096
N_NODES = 4095
NCORES = 8
B_LOC = B // NCORES          # 1024 rows per core

MACRO = 256                  # batch rows per macro tile
SUB = 128                    # rows per subtile (one partition block)
NSUB = MACRO // SUB          # 2
NMACRO = B_LOC // MACRO      # 4
NF = MACRO // 16             # 16-row wrap slots per macro

K_MM = 9                     # levels 0..K_MM-1 handled by one-hot matmul
N_SH = 2 ** K_MM - 1         # shallow nodes (511)
NCHUNK = (N_SH + 127) // 128  # 4
N_GL = DEPTH - K_MM          # gather levels (3)

Y8_SCALE = 128.0             # deep Y rows stored as e3m4 * 128 (max ~15.5)

_CACHE = {}


def _level_of(n):
    lev = 0
    while n >= 2 ** (lev + 1) - 1:
        lev += 1
    return lev


def _host_consts():
    # powT[j, i] = powmat[i, j] = 2^(i-1-j) for j < i  (lhsT of prefix matmul)
    powT = np.zeros((DEPTH, DEPTH), np.float32)
    for i in range(DEPTH):
        for j in range(i):
            powT[j, i] = float(1 << (i - 1 - j))
    # offs_w[p, l*NF+f] = 2^l - 1 (level offset in wrapped (level, slot) layout)
    offs = np.array([(1 << i) - 1 for i in range(DEPTH)], np.float32)
    offs_w = np.broadcast_to(np.repeat(offs, NF)[None, :], (16, DEPTH * NF)).copy()
    # bselT[l, c*128+p] = 1 if level(c*128+p) == l else 0   (lhsT of bc matmul)
    bselT = np.zeros((DEPTH, NCHUNK * 128), np.float32)
    # nrel[p, c] = node - (2^level - 1), or -1 for pad positions
    nrel = np.full((128, NCHUNK), -1.0, np.float32)
    for c in range(NCHUNK):
        for p in range(128):
            n = c * 128 + p
            if n < N_SH:
                lev = _level_of(n)
                bselT[lev, c * 128 + p] = 1.0
                nrel[p, c] = float(n - ((1 << lev) - 1))
    ident = np.eye(DEPTH, dtype=np.float32)
    return powT, offs_w, bselT, nrel, ident


def _build_program():
    import concourse.bass as bass
    import concourse.bacc as bacc
    import concourse.mybir as mybir
    import concourse.tile as tile
    from contextlib import ExitStack

    dt = mybir.dt
    f32 = dt.float32
    bf16 = dt.bfloat16
    f8 = dt.float8e3
    i16 = dt.int16
    u8 = dt.uint8
    Alu = mybir.AluOpType

    nc = bacc.Bacc(trn_type="TRN2", num_swdge_queues=4)

    xt_d = nc.dram_tensor("xt", [NMACRO, 128, 32, MACRO], f32, kind="ExternalInput")
    y8_d = nc.dram_tensor("y8", [N_NODES, D], u8, kind="ExternalInput")
    # zero-padded shallow Y rows (pad rows past N_SH must be 0.0, not garbage:
    # the one-hot matmul multiplies them by 0 and 0*NaN would poison PSUM)
    ysh_d = nc.dram_tensor("ysh", [NCHUNK * 128, D], bf16, kind="ExternalInput")
    wt_d = nc.dram_tensor("wt", [128, 32, DEPTH], f32, kind="ExternalInput")
    powt_d = nc.dram_tensor("powt", [DEPTH, DEPTH], f32, kind="ExternalInput")
    offsw_d = nc.dram_tensor("offsw", [16, DEPTH * NF], f32, kind="ExternalInput")
    bselt_d = nc.dram_tensor("bselt", [DEPTH, NCHUNK * 128], bf16, kind="ExternalInput")
    nrel_d = nc.dram_tensor("nrel", [128, NCHUNK], f32, kind="ExternalInput")
    ident_d = nc.dram_tensor("ident", [DEPTH, DEPTH], f32, kind="ExternalInput")
    out_d = nc.dram_tensor("out", [B_LOC, D], bf16, kind="ExternalOutput")

    with tile.TileContext(nc) as tc, ExitStack() as ctx:
        consts = ctx.enter_context(tc.tile_pool(name="consts", bufs=1))
        xt_p = ctx.enter_context(tc.tile_pool(name="xt", bufs=2))
        small = ctx.enter_context(tc.tile_pool(name="small", bufs=3))
        small4 = ctx.enter_context(tc.tile_pool(name="small4", bufs=4))
        st_p = ctx.enter_context(tc.tile_pool(name="st", bufs=2 * NCHUNK))
        g_p = ctx.enter_context(tc.tile_pool(name="g", bufs=9))
        out_p = ctx.enter_context(tc.tile_pool(name="outp", bufs=3))
        dram_p = ctx.enter_context(tc.tile_pool(name="idxd", bufs=2, space="DRAM"))
        ps_lam = ctx.enter_context(tc.tile_pool(name="pslam", bufs=2, space="PSUM"))
        ps_bc = ctx.enter_context(tc.tile_pool(name="psbc", bufs=2, space="PSUM"))
        ps_tp = ctx.enter_context(tc.tile_pool(name="pstp", bufs=1, space="PSUM"))
        ps_out = ctx.enter_context(tc.tile_pool(name="psout", bufs=2, space="PSUM"))

        # ---- critical-path constant: router weights only ----
        wt_sb = consts.tile([128, 32, DEPTH], f32)
        nc.sync.dma_start(wt_sb[:], wt_d.ap())

        # shallow Y rows: issued up-front on the (idle-until-gathers) Pool
        # software-DGE queue so they don't starve behind the xt stream on the
        # sync HWDGE queue (observed 58GB/s trickle + 90us gather delay when
        # these sat on the scalar HWDGE queue)
        ysh_sb = consts.tile([128, NCHUNK * D], bf16)
        for c in range(NCHUNK):
            nc.gpsimd.dma_start(
                ysh_sb[:, c * D : (c + 1) * D],
                ysh_d.ap()[c * 128 : (c + 1) * 128, :],
            )

        powt_sb = consts.tile([DEPTH, DEPTH], f32)
        offsw_sb = consts.tile([16, DEPTH * NF], f32)
        bselt_sb = consts.tile([DEPTH, NCHUNK * 128], bf16)
        nrel_sb = consts.tile([128, NCHUNK], f32)
        ident_sb = consts.tile([DEPTH, DEPTH], f32)

        lam_state = {}
        state = {}

        def emit_A(m):
            # x^T macro tile in 4 finer DMAs so the router's first K-chunks
            # start before the full 4MB lands
            xt = xt_p.tile([128, 32, MACRO], f32, tag="xt")
            for h in range(4):
                nc.sync.dma_start(
                    xt[:, 8 * h : 8 * (h + 1), :], xt_d.ap()[m][:, 8 * h : 8 * (h + 1), :]
                )
            # router: lam^T [12, MACRO] = W @ x^T  (exact fp32)
            lam_ps = ps_lam.tile([DEPTH, MACRO], f32, tag="lam")
            for c in range(32):
                nc.tensor.matmul(
                    lam_ps[:], wt_sb[:, c, :], xt[:, c, :],
                    start=(c == 0), stop=(c == 31),
                )
            if m == 0:
                nc.sync.dma_start(powt_sb[:], powt_d.ap())
                nc.sync.dma_start(offsw_sb[:], offsw_d.ap())
                nc.sync.dma_start(bselt_sb[:], bselt_d.ap())
                nc.sync.dma_start(nrel_sb[:], nrel_d.ap())
                nc.sync.dma_start(ident_sb[:], ident_d.ap())
            lam_state[m] = lam_ps

        def emit_B(m):
            lam_ps = lam_state.pop(m)
            branch = small.tile([DEPTH, MACRO], f32, tag="branch")
            nc.vector.tensor_scalar(branch[:], lam_ps[:], 0.0, None, Alu.is_gt)
            lamT = small.tile([DEPTH, MACRO], f32, tag="lamT")
            nc.scalar.copy(lamT[:], lam_ps[:])
            lamTb = small.tile([DEPTH, MACRO], bf16, tag="lamTb")
            nc.scalar.copy(lamTb[:], lam_ps[:])

            # prefix^T [12, MACRO] = powmat @ branch  (exact fp32)
            pb_ps = ps_lam.tile([DEPTH, MACRO], f32, tag="lam")
            nc.tensor.matmul(pb_ps[:], powt_sb[:], branch[:], start=True, stop=True)
            pfxT = small.tile([DEPTH, MACRO], f32, tag="pfxT")
            nc.scalar.copy(pfxT[:], pb_ps[:])
            # bf16 copy for the bc matmuls (prefix values <= 255: exact)
            pfxTb = small.tile([DEPTH, MACRO], bf16, tag="pfxTb")
            nc.scalar.copy(pfxTb[:], pb_ps[:])

            # ---- S^T build: one chunk of 128 shallow nodes at a time ----
            st = []
            for c in range(NCHUNK):
                bc_ps = ps_bc.tile([128, 2 * MACRO], f32, tag="bc")
                nc.tensor.matmul(
                    bc_ps[:, :MACRO], bselt_sb[:, c * 128 : (c + 1) * 128],
                    pfxTb[:], start=True, stop=True,
                )
                nc.tensor.matmul(
                    bc_ps[:, MACRO:], bselt_sb[:, c * 128 : (c + 1) * 128],
                    lamTb[:], start=True, stop=True,
                )
                lbc = small.tile([128, MACRO], f32, tag="lbc")
                nc.scalar.copy(lbc[:], bc_ps[:, MACRO:])
                stc = st_p.tile([128, MACRO], bf16, tag="st")
                nc.vector.scalar_tensor_tensor(
                    stc[:], bc_ps[:, :MACRO], nrel_sb[:, c : c + 1], lbc[:],
                    Alu.is_equal, Alu.mult,
                )
                st.append(stc)

            # ---- node ids in the 16-partition-wrapped layout ----
            tpw = ps_tp.tile([16, NF * DEPTH], f32, tag="tpw")
            w_ps = tpw[:].rearrange("p (f l) -> p f l", f=NF)
            for f in range(NF):
                nc.tensor.matmul(
                    w_ps[:, f, :], pfxT[:, f * 16 : (f + 1) * 16], ident_sb[:],
                    start=True, stop=True,
                )
            idx16 = small4.tile([16, DEPTH, NF], i16, tag="idx16")
            nc.vector.tensor_tensor(
                idx16[:], w_ps[:].rearrange("p f l -> p l f"), offsw_sb[:], Alu.add
            )
            # replicate to all 8 Q7 descriptor-gen cores via a DRAM bounce
            # (on sync: tiny transfers, keeps the Pool queue pure gathers)
            idxd = dram_p.tile([16, N_GL * NF], i16, tag="idxd")
            nc.sync.dma_start(
                idxd[:], idx16[:, K_MM:, :].rearrange("p l f -> p (l f)")
            )
            idxr = small4.tile([128, N_GL, NF], i16, tag="idxr")
            for gq in range(8):
                nc.sync.dma_start(
                    idxr[16 * gq : 16 * (gq + 1), :, :].rearrange("p l f -> p (l f)"),
                    idxd[:],
                )

            # per-sub lam scalars, pre-divided by the fp8 storage scale
            lambs = []
            for s in range(NSUB):
                tp2 = ps_tp.tile([SUB, DEPTH], f32, tag="tp2")
                nc.tensor.matmul(
                    tp2[:], lamT[:, s * SUB : (s + 1) * SUB], ident_sb[:],
                    start=True, stop=True,
                )
                lamb = small4.tile([SUB, DEPTH], bf16, tag="lamb")
                nc.vector.tensor_scalar(
                    lamb[:], tp2[:], 1.0 / Y8_SCALE, None, Alu.mult
                )
                lambs.append(lamb)

            state[m] = (st, idxr, lambs)

        def emit_C(m, s):
            st, idxr, lambs = state[m]
            if s == NSUB - 1:
                del state[m]
            bsl = slice(s * SUB, (s + 1) * SUB)
            # ---- gather deep levels from HBM (fp8 rows) ----
            gt = []
            for li in range(N_GL):
                g = g_p.tile([128, 1, D], f8, tag="g")
                nc.gpsimd.dma_gather(
                    g[:], y8_d.ap().bitcast(f8),
                    idxr[:, li, s * (SUB // 16) : (s + 1) * (SUB // 16)],
                    SUB, SUB, D,
                    queue_num=(m * NSUB * N_GL + s * N_GL + li) % 4,
                )
                gt.append(g)

            # ---- shallow one-hot matmul; PSUM->bf16 copies on Act ----
            out_t = out_p.tile([SUB, D], bf16, tag="out")
            for q in range(D // 512):
                qsl = slice(q * 512, (q + 1) * 512)
                po = ps_out.tile([SUB, 512], f32, tag="po")
                for c in range(NCHUNK):
                    nc.tensor.matmul(
                        po[:], st[c][:, bsl],
                        ysh_sb[:, c * D + q * 512 : c * D + (q + 1) * 512],
                        start=(c == 0), stop=(c == NCHUNK - 1),
                    )
                nc.scalar.copy(out_t[:, qsl], po[:])

            # ---- deep levels: per-partition FMA chain on DVE ----
            lamb = lambs[s]
            for li in range(N_GL):
                nc.vector.scalar_tensor_tensor(
                    out_t[:], gt[li][:, 0, :],
                    lamb[:, K_MM + li : K_MM + li + 1],
                    out_t[:], Alu.mult, Alu.add,
                )
            nc.scalar.dma_start(out_d.ap()[m * MACRO + s * SUB :][:SUB, :], out_t[:])

        # ---- software-pipelined emission ----
        emit_A(0)
        emit_B(0)
        emit_A(1)
        for m in range(1, NMACRO):
            emit_C(m - 1, 0)
            emit_B(m)
            emit_C(m - 1, 1)
            if m + 1 < NMACRO:
                emit_A(m + 1)
        emit_C(NMACRO - 1, 0)
        emit_C(NMACRO - 1, 1)

    nc.compile()
    return nc


def _patch_walrus_passes():
    # The default walrus pass list in this environment omits
    # lower_custom_kernel, which the Pool custom instructions (dma_gather)
    # need. Inject it in front of codegen.
    import concourse.bass_utils as bu

    if getattr(bu, "_ant_lck_patched", False):
        return
    bu._ant_lck_patched = True
    orig = bu.run_command

    def run_command(argv, **kw):
        if argv and "walrus_driver" in str(argv[0]):
            argv = list(argv)
            for i, a in enumerate(argv):
                if a == "--pass" and "lower_custom_kernel" not in argv[i + 1]:
                    argv[i + 1] = argv[i + 1].replace(
                        "codegen", "lower_custom_kernel,codegen"
                    )
                    break
        return orig(argv, **kw)

    bu.run_command = run_command


def _get_program():
    if "nc" not in _CACHE:
        _CACHE["nc"] = _build_program()
    return _CACHE["nc"]


def _prep_in_maps(x, W, Y):
    powT, offs_w, bselT, nrel, ident = _host_consts()
    Yf = np.ascontiguousarray(Y, np.float32)
    assert np.abs(Yf).max() * Y8_SCALE <= 15.5, "e3m4 would saturate"
    y8 = (Yf * Y8_SCALE).astype(ml_dtypes.float8_e3m4).view(np.uint8)
    Yb = Yf.astype(ml_dtypes.bfloat16)
    ysh = np.zeros((NCHUNK * 128, D), ml_dtypes.bfloat16)
    ysh[:N_SH] = Yb[:N_SH]
    wt = np.ascontiguousarray(
        W.T.reshape(32, 128, DEPTH).transpose(1, 0, 2), np.float32
    )
    in_maps = []
    xr = x.reshape(NCORES, B_LOC, D)
    for c in range(NCORES):
        xt = xr[c].T  # [D, B_LOC]
        xtm = np.ascontiguousarray(
            xt.reshape(32, 128, NMACRO, MACRO).transpose(2, 1, 0, 3), np.float32
        )
        in_maps.append(
            {
                "xt": xtm, "y8": y8, "ysh": ysh, "wt": wt, "powt": powT,
                "offsw": offs_w, "bselt": bselT.astype(ml_dtypes.bfloat16),
                "nrel": nrel, "ident": ident,
            }
        )
    return in_maps


def kernel(x, W, Y, _trace=False):
    from concourse.bass_utils import run_bass_kernel_spmd

    _patch_walrus_passes()

    nc = _get_program()
    in_maps = _prep_in_maps(np.asarray(x), np.asarray(W), np.asarray(Y))
    res = run_bass_kernel_spmd(nc, in_maps, list(range(NCORES)), trace=_trace)
    out = np.concatenate(
        [np.asarray(res.results[c]["out"], dtype=np.float32) for c in range(NCORES)],
        axis=0,
    )
    if _trace:
        _CACHE["last_result"] = res
    return out


# revision 18
# speedup vs baseline: 1.2165x; 1.0419x over previous
# Trainium2 Bass kernel for nn_FFF_v2 (fast-feedforward / MoE tree routing).
#
#   lam   = x @ W.T                      [B, 12] router logits
#   branch= lam > 0                      tree descent decisions
#   node  = (2^i - 1) + sum_{j<i} branch_j 2^(i-1-j)
#   out   = sum_i lam_i * Y[node_i]      [B, 4096]
#
# Sharding: data-parallel on batch across 8 cores (1024 rows each); W and Y
# replicated.  Per core:
#   - router logits via PE matmul in exact fp32 (branch signs must match the
#     fp32 reference; bf16 here would flip ~300 branches and fail absmax)
#   - tree-node ids via small exact matmuls (powers-of-two weights, fp32)
#   - levels 0..K_MM-1: scaled-one-hot matmul (bf16) against SBUF-resident
#     shallow Y rows -- exploits the massive node reuse at shallow levels
#   - levels K_MM..11: dma_gather of fp8(e3m4, x128) Y rows from HBM +
#     per-partition FMA chain on the vector engine (lam scalars carry /128)
#   - PSUM->SBUF copies of the one-hot result run on the Activation engine
#   - output written in bf16 (halves write traffic); host upcasts to fp32
#
# Schedule: 8 macro stages of 128 rows, software-pipelined with skew 2:
#   iteration m emits  A(m)=xt+router,  C(m-2)=gather+onehot+FMA+store,
#   B(m)=branch/prefix/onehot-build/index.  Per-engine queues stay dense:
#   PE sees router | onehot | small-MM round-robin (clock stays ramped),
#   DVE sees FMA(m-2) then B(m)'s small ops (branch(m) is ready by then),
#   Pool sees a pure gather stream, sync a pure xt+index stream.
import numpy as np
import ml_dtypes

DEPTH = 12
B = 8192
D = 4096
N_NODES = 4095
NCORES = 8
B_LOC = B // NCORES          # 1024 rows per core

MACRO = 128                  # batch rows per macro stage (= one partition block)
NMACRO = B_LOC // MACRO      # 8
NF = MACRO // 16             # 16-row wrap slots per macro (8)

K_MM = 9                     # levels 0..K_MM-1 handled by one-hot matmul
N_SH = 2 ** K_MM - 1         # shallow nodes (511)
NCHUNK = (N_SH + 127) // 128  # 4
N_GL = DEPTH - K_MM          # gather levels (3)

Y8_SCALE = 128.0             # deep Y rows stored as e3m4 * 128 (max ~15.5)

_CACHE = {}


def _level_of(n):
    lev = 0
    while n >= 2 ** (lev + 1) - 1:
        lev += 1
    return lev


def _host_consts():
    # powT[j, i] = powmat[i, j] = 2^(i-1-j) for j < i  (lhsT of prefix matmul)
    powT = np.zeros((DEPTH, DEPTH), np.float32)
    for i in range(DEPTH):
        for j in range(i):
            powT[j, i] = float(1 << (i - 1 - j))
    # offs_w[p, l*NF+f] = 2^l - 1 (level offset in wrapped (level, slot) layout)
    offs = np.array([(1 << i) - 1 for i in range(DEPTH)], np.float32)
    offs_w = np.broadcast_to(np.repeat(offs, NF)[None, :], (16, DEPTH * NF)).copy()
    # bselT[l, c*128+p] = 1 if level(c*128+p) == l else 0   (lhsT of bc matmul)
    bselT = np.zeros((DEPTH, NCHUNK * 128), np.float32)
    # nrel[p, c] = node - (2^level - 1), or -1 for pad positions
    nrel = np.full((128, NCHUNK), -1.0, np.float32)
    for c in range(NCHUNK):
        for p in range(128):
            n = c * 128 + p
            if n < N_SH:
                lev = _level_of(n)
                bselT[lev, c * 128 + p] = 1.0
                nrel[p, c] = float(n - ((1 << lev) - 1))
    ident = np.eye(DEPTH, dtype=np.float32)
    return powT, offs_w, bselT, nrel, ident


def _build_program():
    import concourse.bass as bass
    import concourse.bacc as bacc
    import concourse.mybir as mybir
    import concourse.tile as tile
    from contextlib import ExitStack

    dt = mybir.dt
    f32 = dt.float32
    bf16 = dt.bfloat16
    f8 = dt.float8e3
    i16 = dt.int16
    u8 = dt.uint8
    Alu = mybir.AluOpType

    nc = bacc.Bacc(trn_type="TRN2", num_swdge_queues=4)

    xt_d = nc.dram_tensor("xt", [NMACRO, 128, 32, MACRO], f32, kind="ExternalInput")
    y8_d = nc.dram_tensor("y8", [N_NODES, D], u8, kind="ExternalInput")
    # zero-padded shallow Y rows (pad rows past N_SH must be 0.0, not garbage:
    # the one-hot matmul multiplies them by 0 and 0*NaN would poison PSUM)
    ysh_d = nc.dram_tensor("ysh", [NCHUNK * 128, D], bf16, kind="ExternalInput")
    wt_d = nc.dram_tensor("wt", [128, 32, DEPTH], f32, kind="ExternalInput")
    powt_d = nc.dram_tensor("powt", [DEPTH, DEPTH], f32, kind="ExternalInput")
    offsw_d = nc.dram_tensor("offsw", [16, DEPTH * NF], f32, kind="ExternalInput")
    bselt_d = nc.dram_tensor("bselt", [DEPTH, NCHUNK * 128], bf16, kind="ExternalInput")
    nrel_d = nc.dram_tensor("nrel", [128, NCHUNK], f32, kind="ExternalInput")
    ident_d = nc.dram_tensor("ident", [DEPTH, DEPTH], f32, kind="ExternalInput")
    out_d = nc.dram_tensor("out", [B_LOC, D], bf16, kind="ExternalOutput")

    with tile.TileContext(nc) as tc, ExitStack() as ctx:
        consts = ctx.enter_context(tc.tile_pool(name="consts", bufs=1))
        xt_p = ctx.enter_context(tc.tile_pool(name="xt", bufs=3))
        small = ctx.enter_context(tc.tile_pool(name="small", bufs=3))
        small6 = ctx.enter_context(tc.tile_pool(name="small6", bufs=6))
        st_p = ctx.enter_context(tc.tile_pool(name="st", bufs=3 * NCHUNK))
        g_p = ctx.enter_context(tc.tile_pool(name="g", bufs=9))
        out_p = ctx.enter_context(tc.tile_pool(name="outp", bufs=3))
        dram_p = ctx.enter_context(tc.tile_pool(name="idxd", bufs=3, space="DRAM"))
        ps_lam = ctx.enter_context(tc.tile_pool(name="pslam", bufs=2, space="PSUM"))
        ps_bc = ctx.enter_context(tc.tile_pool(name="psbc", bufs=2, space="PSUM"))
        ps_tp = ctx.enter_context(tc.tile_pool(name="pstp", bufs=1, space="PSUM"))
        ps_out = ctx.enter_context(tc.tile_pool(name="psout", bufs=2, space="PSUM"))

        # ---- critical-path constant: router weights only ----
        wt_sb = consts.tile([128, 32, DEPTH], f32)
        nc.sync.dma_start(wt_sb[:], wt_d.ap())

        # shallow Y rows: issued up-front on the (idle-until-gathers) Pool
        # software-DGE queue so they don't starve behind the xt stream on the
        # sync HWDGE queue (observed 58GB/s trickle + 90us gather delay when
        # these sat on the scalar HWDGE queue)
        ysh_sb = consts.tile([128, NCHUNK * D], bf16)
        for c in range(NCHUNK):
            nc.gpsimd.dma_start(
                ysh_sb[:, c * D : (c + 1) * D],
                ysh_d.ap()[c * 128 : (c + 1) * 128, :],
            )

        powt_sb = consts.tile([DEPTH, DEPTH], f32)
        offsw_sb = consts.tile([16, DEPTH * NF], f32)
        bselt_sb = consts.tile([DEPTH, NCHUNK * 128], bf16)
        nrel_sb = consts.tile([128, NCHUNK], f32)
        ident_sb = consts.tile([DEPTH, DEPTH], f32)

        lam_state = {}
        state = {}
        gq_counter = [0]

        def emit_A(m):
            # x^T macro tile in 4 finer DMAs so the router's first K-chunks
            # start before the full 2MB lands
            xt = xt_p.tile([128, 32, MACRO], f32, tag="xt")
            for h in range(4):
                nc.sync.dma_start(
                    xt[:, 8 * h : 8 * (h + 1), :],
                    xt_d.ap()[m][:, 8 * h : 8 * (h + 1), :],
                )
            # router: lam^T [12, MACRO] = W @ x^T  (exact fp32)
            lam_ps = ps_lam.tile([DEPTH, MACRO], f32, tag="lam")
            for c in range(32):
                nc.tensor.matmul(
                    lam_ps[:], wt_sb[:, c, :], xt[:, c, :],
                    start=(c == 0), stop=(c == 31),
                )
            if m == 0:
                nc.sync.dma_start(powt_sb[:], powt_d.ap())
                nc.sync.dma_start(offsw_sb[:], offsw_d.ap())
                nc.sync.dma_start(bselt_sb[:], bselt_d.ap())
                nc.sync.dma_start(nrel_sb[:], nrel_d.ap())
                nc.sync.dma_start(ident_sb[:], ident_d.ap())
            lam_state[m] = lam_ps

        def emit_B(m):
            lam_ps = lam_state.pop(m)
            branch = small.tile([DEPTH, MACRO], f32, tag="branch")
            nc.vector.tensor_scalar(branch[:], lam_ps[:], 0.0, None, Alu.is_gt)
            lamT = small.tile([DEPTH, MACRO], f32, tag="lamT")
            nc.scalar.copy(lamT[:], lam_ps[:])
            lamTb = small.tile([DEPTH, MACRO], bf16, tag="lamTb")
            nc.scalar.copy(lamTb[:], lam_ps[:])

            # prefix^T [12, MACRO] = powmat @ branch  (exact fp32)
            pb_ps = ps_lam.tile([DEPTH, MACRO], f32, tag="lam")
            nc.tensor.matmul(pb_ps[:], powt_sb[:], branch[:], start=True, stop=True)
            pfxT = small.tile([DEPTH, MACRO], f32, tag="pfxT")
            nc.scalar.copy(pfxT[:], pb_ps[:])
            # bf16 copy for the bc matmuls (prefix values <= 255: exact)
            pfxTb = small.tile([DEPTH, MACRO], bf16, tag="pfxTb")
            nc.scalar.copy(pfxTb[:], pb_ps[:])

            # ---- S^T build: one chunk of 128 shallow nodes at a time ----
            st = []
            for c in range(NCHUNK):
                bc_ps = ps_bc.tile([128, 2 * MACRO], f32, tag="bc")
                nc.tensor.matmul(
                    bc_ps[:, :MACRO], bselt_sb[:, c * 128 : (c + 1) * 128],
                    pfxTb[:], start=True, stop=True,
                )
                nc.tensor.matmul(
                    bc_ps[:, MACRO:], bselt_sb[:, c * 128 : (c + 1) * 128],
                    lamTb[:], start=True, stop=True,
                )
                lbc = small.tile([128, MACRO], f32, tag="lbc")
                nc.scalar.copy(lbc[:], bc_ps[:, MACRO:])
                stc = st_p.tile([128, MACRO], bf16, tag="st")
                nc.vector.scalar_tensor_tensor(
                    stc[:], bc_ps[:, :MACRO], nrel_sb[:, c : c + 1], lbc[:],
                    Alu.is_equal, Alu.mult,
                )
                st.append(stc)

            # ---- node ids in the 16-partition-wrapped layout ----
            tpw = ps_tp.tile([16, NF * DEPTH], f32, tag="tpw")
            w_ps = tpw[:].rearrange("p (f l) -> p f l", f=NF)
            for f in range(NF):
                nc.tensor.matmul(
                    w_ps[:, f, :], pfxT[:, f * 16 : (f + 1) * 16], ident_sb[:],
                    start=True, stop=True,
                )
            idx16 = small6.tile([16, DEPTH, NF], i16, tag="idx16")
            nc.vector.tensor_tensor(
                idx16[:], w_ps[:].rearrange("p f l -> p l f"), offsw_sb[:], Alu.add
            )
            # replicate to all 8 Q7 descriptor-gen cores via a DRAM bounce
            # (on sync: tiny transfers, keeps the Pool queue pure gathers)
            idxd = dram_p.tile([16, N_GL * NF], i16, tag="idxd")
            nc.sync.dma_start(
                idxd[:], idx16[:, K_MM:, :].rearrange("p l f -> p (l f)")
            )
            idxr = small6.tile([128, N_GL, NF], i16, tag="idxr")
            for gq in range(8):
                nc.sync.dma_start(
                    idxr[16 * gq : 16 * (gq + 1), :, :].rearrange("p l f -> p (l f)"),
                    idxd[:],
                )

            # lam in batch-partition layout, pre-divided by the fp8 scale
            tp2 = ps_tp.tile([MACRO, DEPTH], f32, tag="tp2")
            nc.tensor.matmul(tp2[:], lamT[:], ident_sb[:], start=True, stop=True)
            lamb = small6.tile([MACRO, DEPTH], bf16, tag="lamb")
            nc.vector.tensor_scalar(lamb[:], tp2[:], 1.0 / Y8_SCALE, None, Alu.mult)

            state[m] = (st, idxr, lamb)

        def emit_C(m):
            st, idxr, lamb = state.pop(m)
            # ---- gather deep levels from HBM (fp8 rows) ----
            gt = []
            for li in range(N_GL):
                g = g_p.tile([128, 1, D], f8, tag="g")
                nc.gpsimd.dma_gather(
                    g[:], y8_d.ap().bitcast(f8), idxr[:, li, :],
                    MACRO, MACRO, D,
                    queue_num=gq_counter[0] % 4,
                )
                gq_counter[0] += 1
                gt.append(g)

            # ---- shallow one-hot matmul; PSUM->bf16 copies on Act ----
            out_t = out_p.tile([MACRO, D], bf16, tag="out")
            for q in range(D // 512):
                qsl = slice(q * 512, (q + 1) * 512)
                po = ps_out.tile([MACRO, 512], f32, tag="po")
                for c in range(NCHUNK):
                    nc.tensor.matmul(
                        po[:], st[c][:],
                        ysh_sb[:, c * D + q * 512 : c * D + (q + 1) * 512],
                        start=(c == 0), stop=(c == NCHUNK - 1),
                    )
                nc.scalar.copy(out_t[:, qsl], po[:])

            # ---- deep levels: per-partition FMA chain on DVE ----
            for li in range(N_GL):
                nc.vector.scalar_tensor_tensor(
                    out_t[:], gt[li][:, 0, :],
                    lamb[:, K_MM + li : K_MM + li + 1],
                    out_t[:], Alu.mult, Alu.add,
                )
            nc.scalar.dma_start(out_d.ap()[m * MACRO :][:MACRO, :], out_t[:])

        # ---- software-pipelined emission, skew 2 ----
        for m in range(NMACRO):
            emit_A(m)
            if m >= 2:
                emit_C(m - 2)
            emit_B(m)
        emit_C(NMACRO - 2)
        emit_C(NMACRO - 1)

    nc.compile()
    return nc


def _patch_walrus_passes():
    # The default walrus pass list in this environment omits
    # lower_custom_kernel, which the Pool custom instructions (dma_gather)
    # need. Inject it in front of codegen.
    import concourse.bass_utils as bu

    if getattr(bu, "_ant_lck_patched", False):
        return
    bu._ant_lck_patched = True
    orig = bu.run_command

    def run_command(argv, **kw):
        if argv and "walrus_driver" in str(argv[0]):
            argv = list(argv)
            for i, a in enumerate(argv):
                if a == "--pass" and "lower_custom_kernel" not in argv[i + 1]:
                    argv[i + 1] = argv[i + 1].replace(
                        "codegen", "lower_custom_kernel,codegen"
                    )
                    break
        return orig(argv, **kw)

    bu.run_command = run_command


def _get_program():
    if "nc" not in _CACHE:
        _CACHE["nc"] = _build_program()
    return _CACHE["nc"]


def _prep_in_maps(x, W, Y):
    powT, offs_w, bselT, nrel, ident = _host_consts()
    Yf = np.ascontiguousarray(Y, np.float32)
    assert np.abs(Yf).max() * Y8_SCALE <= 15.5, "e3m4 would saturate"
    y8 = (Yf * Y8_SCALE).astype(ml_dtypes.float8_e3m4).view(np.uint8)
    Yb = Yf.astype(ml_dtypes.bfloat16)
    ysh = np.zeros((NCHUNK * 128, D), ml_dtypes.bfloat16)
    ysh[:N_SH] = Yb[:N_SH]
    wt = np.ascontiguousarray(
        W.T.reshape(32, 128, DEPTH).transpose(1, 0, 2), np.float32
    )
    in_maps = []
    xr = x.reshape(NCORES, B_LOC, D)
    for c in range(NCORES):
        xt = xr[c].T  # [D, B_LOC]
        xtm = np.ascontiguousarray(
            xt.reshape(32, 128, NMACRO, MACRO).transpose(2, 1, 0, 3), np.float32
        )
        in_maps.append(
            {
                "xt": xtm, "y8": y8, "ysh": ysh, "wt": wt, "powt": powT,
                "offsw": offs_w, "bselt": bselT.astype(ml_dtypes.bfloat16),
                "nrel": nrel, "ident": ident,
            }
        )
    return in_maps


def kernel(x, W, Y, _trace=False):
    from concourse.bass_utils import run_bass_kernel_spmd

    _patch_walrus_passes()

    nc = _get_program()
    in_maps = _prep_in_maps(np.asarray(x), np.asarray(W), np.asarray(Y))
    res = run_bass_kernel_spmd(nc, in_maps, list(range(NCORES)), trace=_trace)
    out = np.concatenate(
        [np.asarray(res.results[c]["out"], dtype=np.float32) for c in range(NCORES)],
        axis=0,
    )
    if _trace:
        _CACHE["last_result"] = res
    return out
